# revision 2
# baseline (speedup 1.0000x reference)
"""3D Haar wavelet transform (2x2x2, causal temporal pad) on 8 Trainium2 cores.

Input  x: (2, 3, 33, 512, 512) fp32
Output y: (2, 24, 17, 256, 256) fp32   (channel = 3*s + c, s = subband)

Sharding: pure data parallel over H — core ci handles input rows
[64*ci, 64*ci+64) i.e. output rows [32*ci, 32*ci+32).

Memory-bound problem -> move I/O in bf16 (rel err ~2e-3, gate is 2e-2),
halving HBM traffic vs fp32: 13.4 MB in + 13.4 MB out per core
(~75 us roofline at 358 GB/s).

All THREE Haar stages (T, H, W) are fused into a single 128x128 matmul
by packing the 2x2x2 block offsets into the partition dim on the host:
  partition p = i*64 + j*32 + k*16 + r   (i=temporal, j=row, k=col parity,
                                          r = q%16 of the 32 output rows)
  free     f = qh*4352 + T'*256 + w'     (qh = q//16)
Weight W128[p, m] = (-1)^(i*di + j*dj + k*dk) * [r_p == r_m],
  m = di*64 + dj*32 + dk*16 + r  (bf16-exact +-1; scale 0.3536 applied
  on the PSUM->SBUF evacuation, split across DVE and ACT engines).

Per-core device pipeline, per (b, c) slab ([128, 8704] bf16):
  2 in-DMAs (sync/HWDGE, ~1.1 MB each, fully contiguous rows)
  17 matmuls [128x128] x [128, 512] bf16 -> PSUM fp32 (4-bank groups x2)
  5 evacuations x0.3536 -> bf16 staging (alternating ACT / DVE)
  2 out-DMAs (scalar/HWDGE, fully contiguous rows)
Host does all index packing/unpacking (prep/post transposes + bf16 cast).
"""

import numpy as np
import ml_dtypes

import concourse.bacc as bacc
import concourse.mybir as mybir
from concourse import tile
from concourse.bass_utils import run_bass_kernel_spmd

BF16 = ml_dtypes.bfloat16

P = 128
B_, C_, T_, H_, W_ = 2, 3, 33, 512, 512
NCORES = 8
HC = H_ // NCORES          # 64 input rows per core
TP = (T_ + 1) // 2         # 17 output frames
HP = HC // 2               # 32 output rows per core
WP = W_ // 2               # 256 output cols
F = TP * 512               # 8704 free columns per (b, c) slab
FH = 9 * 512               # first-half DMA split (4608)
SCALE = float(np.float32(0.3536))
F32 = mybir.dt.float32
BF16DT = mybir.dt.bfloat16


def _haar_matrix() -> np.ndarray:
    """W128[p, m]: all three Haar sign stages + r-permutation, +-1 entries."""
    W = np.zeros((P, P), dtype=np.float32)
    for i in range(2):
        for j in range(2):
            for k in range(2):
                p0 = i * 64 + j * 32 + k * 16
                for di in range(2):
                    for dj in range(2):
                        for dk in range(2):
                            m0 = di * 64 + dj * 32 + dk * 16
                            sgn = (-1.0) ** (i * di + j * dj + k * dk)
                            for r in range(16):
                                W[p0 + r, m0 + r] = sgn
    return W.astype(BF16)


def build_nc():
    nc = bacc.Bacc("TRN2", target_bir_lowering=False, debug=False)
    x_d = nc.dram_tensor("x", [B_, C_, P, F], BF16DT, kind="ExternalInput")
    y_d = nc.dram_tensor("y", [B_, C_, P, F], BF16DT, kind="ExternalOutput")
    w_d = nc.inline_tensor(_haar_matrix(), name="haar_w")

    with tile.TileContext(nc) as tc:
        with (
            tc.tile_pool(name="wpool", bufs=1) as wpool,
            tc.tile_pool(name="apool", bufs=3) as apool,
            tc.tile_pool(name="cpool", bufs=3) as cpool,
            tc.tile_pool(name="psum", bufs=2, space="PSUM") as psum_pool,
        ):
            w_sb = wpool.tile([P, P], BF16DT)
            nc.sync.dma_start(out=w_sb[:], in_=w_d[:])

            for b in range(B_):
                for c in range(C_):
                    xin = x_d[b, c]
                    yout = y_d[b, c]
                    a = apool.tile([P, F], BF16DT, tag="a")
                    nc.sync.dma_start(out=a[:, :FH], in_=xin[:, :FH])
                    nc.sync.dma_start(out=a[:, FH:], in_=xin[:, FH:])
                    cb = cpool.tile([P, F], BF16DT, tag="c")
                    for gi, g0 in enumerate(range(0, TP, 4)):
                        tg = min(4, TP - g0)
                        ps = psum_pool.tile([P, 2048], F32, tag="ps")
                        for t in range(tg):
                            nc.tensor.matmul(
                                ps[:, t * 512 : (t + 1) * 512],
                                w_sb[:],
                                a[:, (g0 + t) * 512 : (g0 + t + 1) * 512],
                                start=True,
                                stop=True,
                            )
                        src = ps[:, : tg * 512]
                        dst = cb[:, g0 * 512 : (g0 + tg) * 512]
                        # balance evacuation: ACT groups {0,2,4}, DVE {1,3}
                        if gi % 2 == 0:
                            nc.scalar.mul(dst, src, SCALE)
                        else:
                            nc.vector.tensor_scalar_mul(dst, src, SCALE)
                        # drain staging as soon as its groups are complete
                        if g0 + tg == 8:
                            nc.scalar.dma_start(
                                out=yout[:, : 8 * 512], in_=cb[:, : 8 * 512]
                            )
                        elif g0 + tg == TP:
                            nc.scalar.dma_start(
                                out=yout[:, 8 * 512 :], in_=cb[:, 8 * 512 :]
                            )
    nc.compile()
    return nc


_NC_CACHE = None


def _get_nc():
    global _NC_CACHE
    if _NC_CACHE is None:
        _NC_CACHE = build_nc()
    return _NC_CACHE


# xp[tp] = x[max(tp-1, 0)] (causal pad); pair (T', i) reads xp[2T'+i]
_TIDX = np.maximum(np.arange(2 * TP) - 1, 0)


def _prep_core_input(x16: np.ndarray, ci: int) -> np.ndarray:
    """x16 (full input, bf16) -> [B, C, 128, 8704] for core ci."""
    xc = x16[:, :, _TIDX][:, :, :, HC * ci : HC * (ci + 1), :]  # [2,3,34,64,512]
    # h = qh*32 + r*2 + j ; frames -> (T', i) ; w -> (w', k)
    xc = xc.reshape(B_, C_, TP, 2, 2, 16, 2, WP, 2)  # [b,c,T',i,qh,r,j,w',k]
    xc = xc.transpose(0, 1, 3, 6, 8, 5, 4, 2, 7)     # [b,c,i,j,k,r,qh,T',w']
    return np.ascontiguousarray(xc.reshape(B_, C_, P, F))


def _make_in_maps(x: np.ndarray):
    x16 = np.asarray(x, dtype=np.float32).astype(BF16)
    return [{"x": _prep_core_input(x16, ci)} for ci in range(NCORES)]


def kernel(x: np.ndarray) -> np.ndarray:
    assert x.shape == (B_, C_, T_, H_, W_), x.shape
    nc = _get_nc()
    in_maps = _make_in_maps(x)
    res = run_bass_kernel_spmd(nc, in_maps, core_ids=list(range(NCORES)))
    y = np.empty((B_, 8 * C_, TP, H_ // 2, WP), dtype=np.float32)
    for ci in range(NCORES):
        yc = np.asarray(res.results[ci]["y"])            # [2,3,128,8704] bf16
        yc = yc.reshape(B_, C_, 2, 2, 2, 16, 2, TP, WP)  # [b,c,di,dj,dk,r,qh,T',w']
        yc = yc.transpose(0, 2, 3, 4, 1, 7, 6, 5, 8)     # [b,di,dj,dk,c,T',qh,r,w']
        y[:, :, :, HP * ci : HP * (ci + 1), :] = yc.reshape(B_, 8 * C_, TP, HP, WP)
    return y
